# revision 20
# baseline (speedup 1.0000x reference)
"""Trainium2 Bass kernel for nn_Actor_Soft_Attention (gnn_message_passing).

Reference computation (B=65536, IN=128, HID=256, OUT=8):
    agents  = [[x0,x1],[x0,x2]]                 # [B,2,2*IN]
    h_ij    = relu(agents @ W1.T + b1)          # [B,2,HID]
    e_ij    = relu(agents @ W2.T + b2)
    a_ij    = softmax(e_ij, axis=1)             # over the 2 neighbors
    h_i     = sum(a_ij * h_ij, axis=1)          # [B,HID]
    xxx     = relu([h_i, x0,x1,x2] @ W3.T + b3) # [B,HID]
    out     = tanh(xxx @ W4.T + b4)             # [B,OUT]

Sharding: pure data parallel over the batch dim, 8192 rows per core on 8
NeuronCores; weights replicated. Host pre-transposes state to feature-major
layouts so the device does no transposes. The 2-way softmax is computed as
sigmoid(e1-e2).

Precision plan (validated vs reference in numpy, L2 rel err ~8e-3):
  - W1/W2 matmuls run fp8e4m3 DoubleRow (K=256 per pass): x in fp8 and
    weights pre-scaled x16 into fp8 range. The x16 comes back out via the
    free ACT `scale` on the e path; on the h path everything stays x16 and
    host-side W3h/16 absorbs it. b1 is folded out through the max() trick
    (relu(h+b) = max(h,-b)+b) with b3_eff = b3 + W3h @ b1 on the host.
  - W3/W4 matmuls and the direct state path stay bf16 (the output is most
    sensitive to them); accumulation always fp32 in PSUM.
"""

import os
import numpy as np
import ml_dtypes

B, IN, HID, OUT = 65536, 128, 256, 8
NCORES = 8
BS = B // NCORES          # rows per core
NT = 512                  # batch columns per tile
T = BS // NT              # tiles per core

# wall free-dim layout (bf16): w3 [0:1280), w4 [1280:1296)
W3_OFF, W4_OFF, WALL_F = 0, 1280, 1296
# ball layout (f32): -16*b1 [0:2), b2 [2:4), b3eff [4:6), b4 [6]
BALL_F = 7
WS = 16.0


def build_nc():
    from concourse import bacc, mybir
    from concourse import tile as tile_mod

    dt = mybir.dt
    f32, bf16, f8 = dt.float32, dt.bfloat16, dt.float8e4
    AF = mybir.ActivationFunctionType
    ALU = mybir.AluOpType
    DR = mybir.MatmulPerfMode.DoubleRow

    nc = bacc.Bacc("TRN2", target_bir_lowering=False, debug=False)

    xt = nc.declare_dram_parameter("xt", [128, T * 3 * NT], bf16, isOutput=False)
    x8 = nc.declare_dram_parameter("x8", [128, T * 4 * NT], f8, isOutput=False)
    wp = nc.declare_dram_parameter("wpack", [128, WALL_F], bf16, isOutput=False)
    w8p = nc.declare_dram_parameter("w8pack", [128, 2, 512], f8, isOutput=False)
    bp = nc.declare_dram_parameter("bpack", [128, BALL_F], f32, isOutput=False)
    out_d = nc.declare_dram_parameter("out", [OUT, BS], f32, isOutput=True)

    with tile_mod.TileContext(nc) as tc:
        with (
            tc.tile_pool(name="const", bufs=1) as cpool,
            tc.tile_pool(name="xin", bufs=3) as xpool,
            tc.tile_pool(name="act", bufs=2) as apool,
            tc.tile_pool(name="psum", bufs=1, space="PSUM") as ppool,
        ):
            wall = cpool.tile([128, WALL_F], bf16)
            w8w = cpool.tile([128, 2, 512], f8)
            ball = cpool.tile([128, BALL_F], f32)
            nc.scalar.dma_start(
                out=w8w[:].rearrange("p s c -> p (s c)"),
                in_=w8p[:].rearrange("p s c -> p (s c)"),
            )
            nc.scalar.dma_start(out=ball[:], in_=bp[:])
            nc.scalar.dma_start(out=wall[:], in_=wp[:])

            # warm the ACT table with the sigmoid set early
            warm = cpool.tile([128, 32], f32)
            nc.gpsimd.memset(warm[:], 0.0)
            nc.scalar.activation(warm[:], warm[:], AF.Sigmoid)

            def w12dr(w, m):
                # [128, 2, 128] fp8 DoubleRow stationary; w=0 -> W1, w=1 -> W2
                o = w * 256 + m * 128
                return w8w[:, :, o : o + 128]

            def w3c(k, m):
                o = W3_OFF + k * 256 + m * 128
                return wall[:, o : o + 128]

            def w4c(k):
                o = W4_OFF + k * OUT
                return wall[:, o : o + OUT]

            def emit_x3_o(x_prev, hi_prev, tp):
                # ---- xxx = relu(W3h/16 @ hiq + W3s @ x + b3_eff) ----
                xxx = []
                for m in (0, 1):
                    ps = ppool.tile([128, NT], f32, tag="psX", bufs=2)
                    nc.tensor.matmul(
                        ps[:], w3c(0, m), hi_prev[0][:], start=True, stop=False
                    )
                    nc.tensor.matmul(
                        ps[:], w3c(1, m), hi_prev[1][:], start=False, stop=False
                    )
                    for j in range(3):
                        nc.tensor.matmul(
                            ps[:], w3c(2 + j, m),
                            x_prev[:, j * NT : (j + 1) * NT],
                            start=False, stop=(j == 2),
                        )
                    xm = apool.tile([128, NT], bf16, tag="xx")
                    if m == 0:
                        nc.vector.tensor_scalar(
                            xm[:], ps[:], ball[:, 4:5], 0.0, ALU.add, ALU.max
                        )
                    else:
                        nc.scalar.activation(
                            xm[:], ps[:], AF.Relu, bias=ball[:, 5:6]
                        )
                    xxx.append(xm)

                # ---- out = tanh(W4 @ xxx + b4) ----
                psO = ppool.tile([OUT, NT], f32, tag="psO", bufs=1)
                nc.tensor.matmul(psO[:], w4c(0), xxx[0][:], start=True, stop=False)
                nc.tensor.matmul(psO[:], w4c(1), xxx[1][:], start=False, stop=True)
                o_sb = apool.tile([OUT, NT], f32, tag="o")
                nc.scalar.activation(o_sb[:], psO[:], AF.Tanh, bias=ball[:OUT, 6:7])
                nc.sync.dma_start(out=out_d[:, tp * NT : (tp + 1) * NT], in_=o_sb[:])

            carry = None
            for t in range(T):
                x8_sb = xpool.tile([128, 2, 2, NT], f8, tag="x8")
                nc.sync.dma_start(
                    out=x8_sb[:].rearrange("p a s n -> p (a s n)"),
                    in_=x8[:, t * 4 * NT : (t + 1) * 4 * NT],
                )
                x_sb = xpool.tile([128, 3 * NT], bf16, tag="x")
                nc.sync.dma_start(
                    out=x_sb[:], in_=xt[:, t * 3 * NT : (t + 1) * 3 * NT]
                )

                def X(j):
                    return x_sb[:, j * NT : (j + 1) * NT]

                def X8(n):
                    # fp8 pair [x0, x_{1+n}] as [128, 2, NT]
                    return x8_sb[:, n, :, :]

                # ---- e path: psE = 16*(W2 @ agents); r = relu(psE/16 + b2) ----
                rr = []
                for m in (0, 1):
                    ps0 = ppool.tile([128, NT], f32, tag="psE", bufs=3)
                    ps1 = ppool.tile([128, NT], f32, tag="psE", bufs=3)
                    nc.tensor.matmul(
                        ps0[:], w12dr(1, m), X8(0), start=True, stop=True, perf_mode=DR
                    )
                    nc.tensor.matmul(
                        ps1[:], w12dr(1, m), X8(1), start=True, stop=True, perf_mode=DR
                    )
                    r = apool.tile([128, 2 * NT], bf16, tag="r")
                    nc.scalar.activation(
                        r[:, 0:NT], ps0[:], AF.Relu,
                        bias=ball[:, 2 + m : 3 + m], scale=1.0 / WS,
                    )
                    nc.scalar.activation(
                        r[:, NT : 2 * NT], ps1[:], AF.Relu,
                        bias=ball[:, 2 + m : 3 + m], scale=1.0 / WS,
                    )
                    rr.append(r)

                # d = e1 - e2 on the (otherwise idle) GpSimd engine
                dvec = apool.tile([128, 2 * NT], bf16, tag="d")
                for m in (0, 1):
                    nc.gpsimd.tensor_sub(
                        dvec[:, m * NT : (m + 1) * NT],
                        rr[m][:, 0:NT], rr[m][:, NT : 2 * NT],
                    )
                avec = apool.tile([128, 2 * NT], bf16, tag="a")
                nc.scalar.activation(avec[:], dvec[:], AF.Sigmoid)

                # ---- h path (x16 domain, b1 folded out via max trick) ----
                #   h2q = max(psH1, -16b1) = 16*(relu(h2+b1) - b1)
                #   tq  = max(psH0, -16b1) - h2q = 16*(relu(h1+b1)-relu(h2+b1))
                #   hiq = a1*tq + h2q = 16*(h_i - b1)   [b1 absorbed in b3_eff]
                hi = []
                for m in (0, 1):
                    psH0 = ppool.tile([128, NT], f32, tag="psH", bufs=2)
                    psH1 = ppool.tile([128, NT], f32, tag="psH", bufs=2)
                    nc.tensor.matmul(
                        psH0[:], w12dr(0, m), X8(0), start=True, stop=True, perf_mode=DR
                    )
                    nc.tensor.matmul(
                        psH1[:], w12dr(0, m), X8(1), start=True, stop=True, perf_mode=DR
                    )
                    h2q = apool.tile([128, NT], bf16, tag="h2q")
                    nc.vector.tensor_scalar_max(h2q[:], psH1[:], ball[:, m : m + 1])
                    tq = apool.tile([128, NT], bf16, tag="tq")
                    nc.vector.scalar_tensor_tensor(
                        tq[:], psH0[:], ball[:, m : m + 1], h2q[:],
                        ALU.max, ALU.subtract,
                    )
                    u_sb = apool.tile([128, NT], bf16, tag="u")
                    nc.vector.tensor_mul(u_sb[:], avec[:, m * NT : (m + 1) * NT], tq[:])
                    hi_m = apool.tile([128, NT], bf16, tag="hi", bufs=4)
                    nc.gpsimd.tensor_add(hi_m[:], u_sb[:], h2q[:])
                    hi.append(hi_m)

                if carry is not None:
                    emit_x3_o(*carry)
                carry = (x_sb, hi, t)
            emit_x3_o(*carry)

    nc.compile()
    return nc


_NC_CACHE = None


def _get_nc():
    global _NC_CACHE
    if _NC_CACHE is None:
        _NC_CACHE = build_nc()
    return _NC_CACHE


def _prep_in_maps(state, W1, b1, W2, b2, W3, b3, W4, b4):
    bf16 = ml_dtypes.bfloat16
    f8 = ml_dtypes.float8_e4m3fn
    state = np.asarray(state, np.float32).reshape(B, 3, IN)
    W3f = np.asarray(W3, np.float32)
    b1f = np.asarray(b1, np.float32)

    def chunks(W, k):
        Wt = np.asarray(W, np.float32).T
        return np.concatenate([Wt[i * 128 : (i + 1) * 128] for i in range(k)], axis=1)

    W3s = W3f.copy()
    W3s[:, :HID] /= WS               # absorbs the x16 scale of hiq
    wpack = np.concatenate([chunks(W3s, 5), chunks(np.asarray(W4, np.float32), 2)],
                           axis=1).astype(bf16)
    assert wpack.shape == (128, WALL_F)

    # fp8 DoubleRow stationary: w8[p, s, w*256 + m*128 + c] = 16*W.T[s*128+p, m*128+c]
    w8pack = np.zeros((128, 2, 512), np.float32)
    for wi, W in ((0, W1), (1, W2)):
        Wt = np.asarray(W, np.float32).T * WS
        for s in (0, 1):
            w8pack[:, s, wi * 256 : wi * 256 + 256] = Wt[s * 128 : (s + 1) * 128]
    w8pack = w8pack.astype(f8)

    b3_eff = np.asarray(b3, np.float32) + W3f[:, :HID] @ b1f
    bpack = np.zeros((128, BALL_F), np.float32)
    bpack[:, 0:2] = -WS * b1f.reshape(2, 128).T
    bpack[:, 2:4] = np.asarray(b2, np.float32).reshape(2, 128).T
    bpack[:, 4:6] = b3_eff.reshape(2, 128).T
    bpack[:OUT, 6] = np.asarray(b4, np.float32)

    in_maps = []
    for c in range(NCORES):
        shard = state[c * BS : (c + 1) * BS]              # [BS, 3, 128]
        perm = shard.reshape(T, NT, 3, IN).transpose(3, 0, 2, 1)  # [128, T, 3, NT]
        xtc = np.ascontiguousarray(perm.reshape(IN, T * 3 * NT)).astype(bf16)
        p8 = perm.astype(f8)                               # [128, T, 3, NT]
        x8c = np.empty((IN, T, 4, NT), f8)
        x8c[:, :, 0] = p8[:, :, 0]
        x8c[:, :, 1] = p8[:, :, 1]
        x8c[:, :, 2] = p8[:, :, 0]
        x8c[:, :, 3] = p8[:, :, 2]
        x8c = np.ascontiguousarray(x8c.reshape(IN, T * 4 * NT))
        in_maps.append({"xt": xtc, "x8": x8c, "wpack": wpack,
                        "w8pack": w8pack, "bpack": bpack})
    return in_maps


def _ensure_ntff_hook():
    """Register the axon NTFF profile hook if the image's antenv lacks it."""
    import sys, types
    try:
        from antenv.axon_hooks import get_axon_ntff_profile_hook  # noqa: F401
        return
    except ImportError:
        pass
    from trn_agent_boot.trn_boot import _ntff_profile_via_ctypes
    hook = _ntff_profile_via_ctypes("/opt/axon/libaxon_pjrt.so")
    mod = types.ModuleType("antenv.axon_hooks")
    holder = {"hook": hook}
    mod.get_axon_ntff_profile_hook = lambda: holder["hook"]
    mod.set_axon_ntff_profile_hook = lambda h: holder.__setitem__("hook", h)
    sys.modules["antenv.axon_hooks"] = mod
    import antenv
    antenv.axon_hooks = mod


def run(inputs, trace=False):
    """Compile (cached), run on 8 cores, return (full_output, exec_time_ns)."""
    from concourse import bass_utils
    from concourse.bass_utils import run_bass_kernel_spmd

    if trace:
        _ensure_ntff_hook()
        bass_utils.upload_artifacts = lambda tmpdir: tmpdir  # no S3 here

    nc = _get_nc()
    in_maps = _prep_in_maps(**inputs)
    res = run_bass_kernel_spmd(nc, in_maps, core_ids=list(range(NCORES)), trace=trace)
    out = np.concatenate([res.results[c]["out"].T for c in range(NCORES)], axis=0)
    return np.ascontiguousarray(out, dtype=np.float32), res.exec_time_ns


def kernel(**inputs) -> np.ndarray:
    out, _ = run(inputs, trace=bool(os.environ.get("KERNEL_TRACE")))
    return out


# revision 22
# speedup vs baseline: 1.1495x; 1.1495x over previous
"""Trainium2 Bass kernel for nn_Actor_Soft_Attention (gnn_message_passing).

Reference computation (B=65536, IN=128, HID=256, OUT=8):
    agents  = [[x0,x1],[x0,x2]]                 # [B,2,2*IN]
    h_ij    = relu(agents @ W1.T + b1)          # [B,2,HID]
    e_ij    = relu(agents @ W2.T + b2)
    a_ij    = softmax(e_ij, axis=1)             # over the 2 neighbors
    h_i     = sum(a_ij * h_ij, axis=1)          # [B,HID]
    xxx     = relu([h_i, x0,x1,x2] @ W3.T + b3) # [B,HID]
    out     = tanh(xxx @ W4.T + b4)             # [B,OUT]

Sharding: pure data parallel over the batch dim, 8192 rows per core on 8
NeuronCores; weights replicated. Host pre-transposes state to feature-major
layouts so the device does no transposes. The 2-way softmax is computed as
sigmoid(e1-e2).

Precision plan (validated vs reference in numpy, L2 rel err ~8e-3):
  - W1/W2 matmuls run fp8e4m3 DoubleRow (K=256 per pass): x in fp8 and
    weights pre-scaled x16 into fp8 range. The x16 comes back out via the
    free ACT `scale` on the e path; on the h path everything stays x16 and
    host-side W3h/16 absorbs it. b1 is folded out through the max() trick
    (relu(h+b) = max(h,-b)+b) with b3_eff = b3 + W3h @ b1 on the host.
  - W3/W4 matmuls and the direct state path stay bf16 (the output is most
    sensitive to them); accumulation always fp32 in PSUM.
"""

import os
import numpy as np
import ml_dtypes

B, IN, HID, OUT = 65536, 128, 256, 8
NCORES = 8
BS = B // NCORES          # rows per core
NT = 512                  # batch columns per tile
T = BS // NT              # tiles per core

# wall free-dim layout (bf16): w3 [0:1280), w4 [1280:1296)
W3_OFF, W4_OFF, WALL_F = 0, 1280, 1296
# ball layout (f32): -16*b1 [0:2), b2 [2:4), b3eff [4:6), b4 [6]
BALL_F = 7
WS = 16.0


def build_nc():
    from concourse import bacc, mybir
    from concourse import tile as tile_mod

    dt = mybir.dt
    f32, bf16, f8 = dt.float32, dt.bfloat16, dt.float8e4
    AF = mybir.ActivationFunctionType
    ALU = mybir.AluOpType
    DR = mybir.MatmulPerfMode.DoubleRow

    nc = bacc.Bacc("TRN2", target_bir_lowering=False, debug=False)

    xt = nc.declare_dram_parameter("xt", [128, T * 3 * NT], bf16, isOutput=False)
    x8 = nc.declare_dram_parameter("x8", [128, T * 4 * NT], f8, isOutput=False)
    wp = nc.declare_dram_parameter("wpack", [128, WALL_F], bf16, isOutput=False)
    w8p = nc.declare_dram_parameter("w8pack", [128, 2, 512], f8, isOutput=False)
    bp = nc.declare_dram_parameter("bpack", [128, BALL_F], f32, isOutput=False)
    out_d = nc.declare_dram_parameter("out", [OUT, BS], f32, isOutput=True)

    with tile_mod.TileContext(nc) as tc:
        with (
            tc.tile_pool(name="const", bufs=1) as cpool,
            tc.tile_pool(name="xin", bufs=3) as xpool,
            tc.tile_pool(name="act", bufs=2) as apool,
            tc.tile_pool(name="psum", bufs=1, space="PSUM") as ppool,
        ):
            wall = cpool.tile([128, WALL_F], bf16)
            w8w = cpool.tile([128, 2, 512], f8)
            ball = cpool.tile([128, BALL_F], f32)
            nc.scalar.dma_start(
                out=w8w[:].rearrange("p s c -> p (s c)"),
                in_=w8p[:].rearrange("p s c -> p (s c)"),
            )
            nc.scalar.dma_start(out=ball[:], in_=bp[:])
            nc.scalar.dma_start(out=wall[:], in_=wp[:])

            # warm the ACT table with the sigmoid set early
            warm = cpool.tile([128, 32], f32)
            nc.gpsimd.memset(warm[:], 0.0)
            nc.scalar.activation(warm[:], warm[:], AF.Sigmoid)

            def w12dr(w, m):
                # [128, 2, 128] fp8 DoubleRow stationary; w=0 -> W1, w=1 -> W2
                o = w * 256 + m * 128
                return w8w[:, :, o : o + 128]

            def w3c(k, m):
                o = W3_OFF + k * 256 + m * 128
                return wall[:, o : o + 128]

            def w4c(k):
                o = W4_OFF + k * OUT
                return wall[:, o : o + OUT]

            def emit_x3_o(x_prev, hi_prev, tp):
                # ---- xxx = relu(W3h/16 @ hiq + W3s @ x + b3_eff) ----
                xxx = []
                for m in (0, 1):
                    ps = ppool.tile([128, NT], f32, tag="psX", bufs=2)
                    nc.tensor.matmul(
                        ps[:], w3c(0, m), hi_prev[0][:], start=True, stop=False
                    )
                    nc.tensor.matmul(
                        ps[:], w3c(1, m), hi_prev[1][:], start=False, stop=False
                    )
                    for j in range(3):
                        nc.tensor.matmul(
                            ps[:], w3c(2 + j, m),
                            x_prev[:, j * NT : (j + 1) * NT],
                            start=False, stop=(j == 2),
                        )
                    xm = apool.tile([128, NT], bf16, tag="xx")
                    if m == 0 and tp % 2 == 0:
                        nc.vector.tensor_scalar(
                            xm[:], ps[:], ball[:, 4:5], 0.0, ALU.add, ALU.max
                        )
                    else:
                        nc.scalar.activation(
                            xm[:], ps[:], AF.Relu, bias=ball[:, 4 + m : 5 + m]
                        )
                    xxx.append(xm)

                # ---- out = tanh(W4 @ xxx + b4) ----
                psO = ppool.tile([OUT, NT], f32, tag="psO", bufs=1)
                nc.tensor.matmul(psO[:], w4c(0), xxx[0][:], start=True, stop=False)
                nc.tensor.matmul(psO[:], w4c(1), xxx[1][:], start=False, stop=True)
                o_sb = apool.tile([OUT, NT], f32, tag="o")
                nc.scalar.activation(o_sb[:], psO[:], AF.Tanh, bias=ball[:OUT, 6:7])
                nc.sync.dma_start(out=out_d[:, tp * NT : (tp + 1) * NT], in_=o_sb[:])

            carry = None
            for t in range(T):
                x8_sb = xpool.tile([128, 2, 2, NT], f8, tag="x8")
                nc.sync.dma_start(
                    out=x8_sb[:].rearrange("p a s n -> p (a s n)"),
                    in_=x8[:, t * 4 * NT : (t + 1) * 4 * NT],
                )
                x_sb = xpool.tile([128, 3 * NT], bf16, tag="x")
                nc.sync.dma_start(
                    out=x_sb[:], in_=xt[:, t * 3 * NT : (t + 1) * 3 * NT]
                )

                def X(j):
                    return x_sb[:, j * NT : (j + 1) * NT]

                def X8(n):
                    # fp8 pair [x0, x_{1+n}] as [128, 2, NT]
                    return x8_sb[:, n, :, :]

                # ---- e path: psE = 16*(W2 @ agents); r = relu(psE/16 + b2) ----
                rr = []
                for m in (0, 1):
                    ps0 = ppool.tile([128, NT], f32, tag="psE", bufs=3)
                    ps1 = ppool.tile([128, NT], f32, tag="psE", bufs=3)
                    nc.tensor.matmul(
                        ps0[:], w12dr(1, m), X8(0), start=True, stop=True, perf_mode=DR
                    )
                    nc.tensor.matmul(
                        ps1[:], w12dr(1, m), X8(1), start=True, stop=True, perf_mode=DR
                    )
                    r = apool.tile([128, 2 * NT], bf16, tag="r")
                    nc.scalar.activation(
                        r[:, 0:NT], ps0[:], AF.Relu,
                        bias=ball[:, 2 + m : 3 + m], scale=1.0 / WS,
                    )
                    nc.scalar.activation(
                        r[:, NT : 2 * NT], ps1[:], AF.Relu,
                        bias=ball[:, 2 + m : 3 + m], scale=1.0 / WS,
                    )
                    rr.append(r)

                # d = e1 - e2 on the (otherwise idle) GpSimd engine
                dvec = apool.tile([128, 2 * NT], bf16, tag="d")
                for m in (0, 1):
                    nc.gpsimd.tensor_sub(
                        dvec[:, m * NT : (m + 1) * NT],
                        rr[m][:, 0:NT], rr[m][:, NT : 2 * NT],
                    )
                avec = apool.tile([128, 2 * NT], bf16, tag="a")
                nc.scalar.activation(avec[:], dvec[:], AF.Sigmoid)

                # ---- h path (x16 domain, b1 folded out via max trick) ----
                #   h2q = max(psH1, -16b1) = 16*(relu(h2+b1) - b1)
                #   tq  = max(psH0, -16b1) - h2q = 16*(relu(h1+b1)-relu(h2+b1))
                #   hiq = a1*tq + h2q = 16*(h_i - b1)   [b1 absorbed in b3_eff]
                hi = []
                for m in (0, 1):
                    psH0 = ppool.tile([128, NT], f32, tag="psH", bufs=2)
                    psH1 = ppool.tile([128, NT], f32, tag="psH", bufs=2)
                    nc.tensor.matmul(
                        psH0[:], w12dr(0, m), X8(0), start=True, stop=True, perf_mode=DR
                    )
                    nc.tensor.matmul(
                        psH1[:], w12dr(0, m), X8(1), start=True, stop=True, perf_mode=DR
                    )
                    h2q = apool.tile([128, NT], bf16, tag="h2q")
                    nc.vector.tensor_scalar_max(h2q[:], psH1[:], ball[:, m : m + 1])
                    tq = apool.tile([128, NT], bf16, tag="tq")
                    nc.vector.scalar_tensor_tensor(
                        tq[:], psH0[:], ball[:, m : m + 1], h2q[:],
                        ALU.max, ALU.subtract,
                    )
                    u_sb = apool.tile([128, NT], bf16, tag="u")
                    nc.vector.tensor_mul(u_sb[:], avec[:, m * NT : (m + 1) * NT], tq[:])
                    hi_m = apool.tile([128, NT], bf16, tag="hi", bufs=4)
                    nc.vector.tensor_add(hi_m[:], u_sb[:], h2q[:])
                    hi.append(hi_m)

                if carry is not None:
                    emit_x3_o(*carry)
                carry = (x_sb, hi, t)
            emit_x3_o(*carry)

    nc.compile()
    return nc


_NC_CACHE = None


def _get_nc():
    global _NC_CACHE
    if _NC_CACHE is None:
        _NC_CACHE = build_nc()
    return _NC_CACHE


def _prep_in_maps(state, W1, b1, W2, b2, W3, b3, W4, b4):
    bf16 = ml_dtypes.bfloat16
    f8 = ml_dtypes.float8_e4m3fn
    state = np.asarray(state, np.float32).reshape(B, 3, IN)
    W3f = np.asarray(W3, np.float32)
    b1f = np.asarray(b1, np.float32)

    def chunks(W, k):
        Wt = np.asarray(W, np.float32).T
        return np.concatenate([Wt[i * 128 : (i + 1) * 128] for i in range(k)], axis=1)

    W3s = W3f.copy()
    W3s[:, :HID] /= WS               # absorbs the x16 scale of hiq
    wpack = np.concatenate([chunks(W3s, 5), chunks(np.asarray(W4, np.float32), 2)],
                           axis=1).astype(bf16)
    assert wpack.shape == (128, WALL_F)

    # fp8 DoubleRow stationary: w8[p, s, w*256 + m*128 + c] = 16*W.T[s*128+p, m*128+c]
    w8pack = np.zeros((128, 2, 512), np.float32)
    for wi, W in ((0, W1), (1, W2)):
        Wt = np.asarray(W, np.float32).T * WS
        for s in (0, 1):
            w8pack[:, s, wi * 256 : wi * 256 + 256] = Wt[s * 128 : (s + 1) * 128]
    w8pack = w8pack.astype(f8)

    b3_eff = np.asarray(b3, np.float32) + W3f[:, :HID] @ b1f
    bpack = np.zeros((128, BALL_F), np.float32)
    bpack[:, 0:2] = -WS * b1f.reshape(2, 128).T
    bpack[:, 2:4] = np.asarray(b2, np.float32).reshape(2, 128).T
    bpack[:, 4:6] = b3_eff.reshape(2, 128).T
    bpack[:OUT, 6] = np.asarray(b4, np.float32)

    in_maps = []
    for c in range(NCORES):
        shard = state[c * BS : (c + 1) * BS]              # [BS, 3, 128]
        perm = shard.reshape(T, NT, 3, IN).transpose(3, 0, 2, 1)  # [128, T, 3, NT]
        xtc = np.ascontiguousarray(perm.reshape(IN, T * 3 * NT)).astype(bf16)
        p8 = perm.astype(f8)                               # [128, T, 3, NT]
        x8c = np.empty((IN, T, 4, NT), f8)
        x8c[:, :, 0] = p8[:, :, 0]
        x8c[:, :, 1] = p8[:, :, 1]
        x8c[:, :, 2] = p8[:, :, 0]
        x8c[:, :, 3] = p8[:, :, 2]
        x8c = np.ascontiguousarray(x8c.reshape(IN, T * 4 * NT))
        in_maps.append({"xt": xtc, "x8": x8c, "wpack": wpack,
                        "w8pack": w8pack, "bpack": bpack})
    return in_maps


def _ensure_ntff_hook():
    """Register the axon NTFF profile hook if the image's antenv lacks it."""
    import sys, types
    try:
        from antenv.axon_hooks import get_axon_ntff_profile_hook  # noqa: F401
        return
    except ImportError:
        pass
    from trn_agent_boot.trn_boot import _ntff_profile_via_ctypes
    hook = _ntff_profile_via_ctypes("/opt/axon/libaxon_pjrt.so")
    mod = types.ModuleType("antenv.axon_hooks")
    holder = {"hook": hook}
    mod.get_axon_ntff_profile_hook = lambda: holder["hook"]
    mod.set_axon_ntff_profile_hook = lambda h: holder.__setitem__("hook", h)
    sys.modules["antenv.axon_hooks"] = mod
    import antenv
    antenv.axon_hooks = mod


def run(inputs, trace=False):
    """Compile (cached), run on 8 cores, return (full_output, exec_time_ns)."""
    from concourse import bass_utils
    from concourse.bass_utils import run_bass_kernel_spmd

    if trace:
        _ensure_ntff_hook()
        bass_utils.upload_artifacts = lambda tmpdir: tmpdir  # no S3 here

    nc = _get_nc()
    in_maps = _prep_in_maps(**inputs)
    res = run_bass_kernel_spmd(nc, in_maps, core_ids=list(range(NCORES)), trace=trace)
    out = np.concatenate([res.results[c]["out"].T for c in range(NCORES)], axis=0)
    return np.ascontiguousarray(out, dtype=np.float32), res.exec_time_ns


def kernel(**inputs) -> np.ndarray:
    out, _ = run(inputs, trace=bool(os.environ.get("KERNEL_TRACE")))
    return out
